# revision 1
# baseline (speedup 1.0000x reference)
"""Trainium2 Bass kernel for ModLinear forward:

    alpha = z @ weight_alpha.T + bias_alpha          # [B, IN]
    beta  = z @ weight_beta.T  + bias_beta           # [B, OUT]
    out   = (x * alpha[:, None, :]) @ weight.T + beta[:, None, :]

Key restructuring: alpha modulates the *input channels*, so it folds into the
weight per batch:  out[b] = x[b] @ (weight.T * alpha[b][:, None]) + beta[b].
The huge x tensor is then consumed by a plain matmul with a per-batch
pre-modulated weight (tiny, computed on host along with alpha/beta).

Sharding: x is flattened to [B*N, IN] and split into 8 contiguous row blocks
(one per NeuronCore); batch boundary falls exactly between cores 3 and 4, so
each core uses a single (wmodT, beta) pair. No cross-core communication.

Device kernel per core (rows = 32768), 1024-row superblocks packed 8 rows
per partition so each DMA moves 2 MiB with 16 KiB descriptors (HBM at peak):
  for each superblock:
    DMA x [128, 8x512] f32 -> SBUF (one 2 MiB load)
    4 passes over row-pairs:
      8x PE-transpose 128x128 -> PSUM  (feature dim onto partitions)
      ACT copy PSUM -> SBUF (xT chunks)
      8x PE matmul (f32r @ full speed, accumulate 4 feature chunks) -> PSUM
      DVE add beta (pre-replicated on host) -> SBUF staging
    DMA out [128, 8x512] -> DRAM (one 2 MiB store, second HWDGE ring)
Measured: best 373.8 us/core on hw (~99% of the 358 GB/s per-core HBM
roofline for the 128 MiB/core traffic, +-5-10% machine variance);
rel err vs fp32 reference ~1.3e-4 (f32r truncation).
"""

import numpy as np

B, N = 2, 131072
IN_F, OUT_F, STYLE_F = 512, 512, 256
NCORES = 8
ROWS = B * N
ROWS_PER_CORE = ROWS // NCORES  # 32768
P = 128


def _build_body(tc, out_ap, x_ap, wt_ap, betar_ap, ident_ap, rows_per_core):
    import concourse.bass as bass
    from concourse import mybir

    nc = tc.nc
    f32 = mybir.dt.float32
    f32r = mybir.dt.float32r
    # Superblock: V*128 rows, packed as [128 partitions, V rows x 512 feat].
    # Partition u holds DRAM rows (V*u .. V*u+V-1) -> V*2 KiB contiguous per
    # partition -> big DMA descriptors, V*512 KiB per dma_start.
    V = 8
    SB = V * P
    nsuper = rows_per_core // SB

    x_v = x_ap.rearrange("(s u v) i -> s u (v i)", u=P, v=V)
    out_v = out_ap.rearrange("(s u v) o -> s u (v o)", u=P, v=V)

    with (
        tc.tile_pool(name="const", bufs=1) as cpool,
        tc.tile_pool(name="xin", bufs=2) as xpool,
        tc.tile_pool(name="xt", bufs=4) as xtpool,
        tc.tile_pool(name="oout", bufs=2) as opool,
        tc.tile_pool(name="ptr", bufs=2, space="PSUM") as ptpool,
        tc.tile_pool(name="pmm", bufs=2, space="PSUM") as pmpool,
    ):
        # Constants: 128x128 identity FIRST (64 KiB; it gates every PE
        # transpose), then modulated transposed weight (4 chunks of
        # [128, 512] side by side) and replicated beta.
        ident_sb = cpool.tile([P, P], f32r)
        nc.sync.dma_start(out=ident_sb[:], in_=ident_ap[:, :])
        wt_sb = cpool.tile([P, 4 * OUT_F], f32r)
        for c in range(4):
            nc.sync.dma_start(
                out=wt_sb[:, c * OUT_F : (c + 1) * OUT_F],
                in_=wt_ap[c * P : (c + 1) * P, :],
            )
        beta_sb = cpool.tile([P, OUT_F], f32)
        nc.sync.dma_start(out=beta_sb[:], in_=betar_ap[:, :])

        for s in range(nsuper):
            xt = xpool.tile([P, V * IN_F], f32r)
            if s == 0:
                # Head of pipeline: split the first 2 MiB load into 512 KiB
                # quarters so the first transposes start ~4 us earlier.
                Q = 2 * IN_F
                for q in range(V // 2):
                    nc.sync.dma_start(
                        out=xt[:, q * Q : (q + 1) * Q],
                        in_=x_v[s][:, q * Q : (q + 1) * Q],
                    )
            else:
                nc.sync.dma_start(out=xt[:], in_=x_v[s])
            ot = opool.tile([P, V * OUT_F], f32)

            # 4 passes of 2 row-groups each (PSUM: 2+2 banks, double-buffered)
            for h in range(V // 2):
                pt = ptpool.tile([P, 2 * IN_F], f32r)
                for gg in range(2):
                    v = 2 * h + gg
                    for c in range(4):
                        nc.tensor.transpose(
                            pt[:, gg * IN_F + c * P : gg * IN_F + (c + 1) * P],
                            xt[:, v * IN_F + c * P : v * IN_F + (c + 1) * P],
                            ident_sb[:],
                        )
                xts = xtpool.tile([P, 2 * IN_F], f32r)
                nc.scalar.copy(out=xts[:, :IN_F], in_=pt[:, :IN_F])
                nc.scalar.copy(out=xts[:, IN_F:], in_=pt[:, IN_F:])

                po = pmpool.tile([P, 2 * OUT_F], f32)
                for gg in range(2):
                    for c in range(4):
                        nc.tensor.matmul(
                            po[:, gg * OUT_F : (gg + 1) * OUT_F],
                            xts[:, gg * IN_F + c * P : gg * IN_F + (c + 1) * P],
                            wt_sb[:, c * OUT_F : (c + 1) * OUT_F],
                            start=(c == 0),
                            stop=(c == 3),
                        )

                for gg in range(2):
                    v = 2 * h + gg
                    nc.vector.tensor_add(
                        out=ot[:, v * OUT_F : (v + 1) * OUT_F],
                        in0=po[:, gg * OUT_F : (gg + 1) * OUT_F],
                        in1=beta_sb[:],
                    )

            nc.scalar.dma_start(out=out_v[s], in_=ot[:])


def build_nc(rows_per_core=ROWS_PER_CORE):
    """Build + compile the per-core Bass program. Returns (nc, names)."""
    import concourse.tile as tile
    from concourse import bacc, mybir

    f32 = mybir.dt.float32
    f32r = mybir.dt.float32r
    nc = bacc.Bacc(
        "TRN2", target_bir_lowering=False, debug=False, num_devices=NCORES
    )
    x_t = nc.dram_tensor("x", [rows_per_core, IN_F], f32r, kind="ExternalInput")
    wt_t = nc.dram_tensor("wt", [IN_F, OUT_F], f32r, kind="ExternalInput")
    betar_t = nc.dram_tensor("betar", [P, OUT_F], f32, kind="ExternalInput")
    ident_t = nc.dram_tensor("ident", [P, P], f32r, kind="ExternalInput")
    out_t = nc.dram_tensor("out", [rows_per_core, OUT_F], f32, kind="ExternalOutput")

    with tile.TileContext(nc) as tc:
        _build_body(
            tc, out_t.ap(), x_t.ap(), wt_t.ap(), betar_t.ap(), ident_t.ap(),
            rows_per_core,
        )
    nc.compile()
    return nc


_NC_CACHE = {}


def _get_nc(rows_per_core=ROWS_PER_CORE):
    if rows_per_core not in _NC_CACHE:
        _NC_CACHE[rows_per_core] = build_nc(rows_per_core)
    return _NC_CACHE[rows_per_core]


def host_prep(x, z, weight, weight_alpha, bias_alpha, weight_beta, bias_beta):
    """Compute per-batch modulated weights + biases, and per-core in_maps."""
    z64 = z.astype(np.float64)
    alpha = (z64 @ weight_alpha.astype(np.float64).T) + bias_alpha.astype(np.float64)
    beta = (z64 @ weight_beta.astype(np.float64).T) + bias_beta.astype(np.float64)
    alpha = alpha.astype(np.float32)  # [B, IN_F]
    beta = beta.astype(np.float32)  # [B, OUT_F]

    # wmodT[b][i, o] = weight[o, i] * alpha[b, i]
    wmodT = [
        np.ascontiguousarray(weight.T * alpha[b][:, None]).astype(np.float32)
        for b in range(B)
    ]
    betar = [
        np.ascontiguousarray(np.broadcast_to(beta[b], (P, OUT_F))).astype(np.float32)
        for b in range(B)
    ]
    ident = np.eye(P, dtype=np.float32)

    xf = np.ascontiguousarray(x).reshape(ROWS, IN_F)
    in_maps = []
    for k in range(NCORES):
        b = (k * ROWS_PER_CORE) // N  # batch this core's rows belong to
        in_maps.append(
            {
                "x": xf[k * ROWS_PER_CORE : (k + 1) * ROWS_PER_CORE],
                "wt": wmodT[b],
                "betar": betar[b],
                "ident": ident,
            }
        )
    return in_maps


def kernel(x, z, weight, weight_alpha, bias_alpha, weight_beta, bias_beta,
           _trace=False):
    from concourse.bass_utils import run_bass_kernel_spmd

    x = np.asarray(x, dtype=np.float32)
    z = np.asarray(z, dtype=np.float32)
    weight = np.asarray(weight, dtype=np.float32)
    weight_alpha = np.asarray(weight_alpha, dtype=np.float32)
    bias_alpha = np.asarray(bias_alpha, dtype=np.float32)
    weight_beta = np.asarray(weight_beta, dtype=np.float32)
    bias_beta = np.asarray(bias_beta, dtype=np.float32)
    in_maps = host_prep(
        x, z, weight, weight_alpha, bias_alpha, weight_beta, bias_beta
    )
    nc = _get_nc()
    res = run_bass_kernel_spmd(
        nc, in_maps, core_ids=list(range(NCORES)), trace=_trace
    )
    out = np.concatenate([res.results[k]["out"] for k in range(NCORES)], axis=0)
    out = out.reshape(B, N, OUT_F)
    if _trace:
        kernel.last_results = res
    return out



# revision 2
# speedup vs baseline: 1.0259x; 1.0259x over previous
"""Trainium2 Bass kernel for ModLinear forward — bf16, weight-stationary.

    alpha = z @ weight_alpha.T + bias_alpha          # [B, IN]
    beta  = z @ weight_beta.T  + bias_beta           # [B, OUT]
    out   = (x * alpha[:, None, :]) @ weight.T + beta[:, None, :]

alpha folds into the weight per batch: out[b] = x[b] @ wmodT[b] + beta[b]
with wmodT[b] = weight.T * alpha[b][:, None]  (tiny, host-computed).

Numerics: the 2e-2 rel-err gate admits bf16 I/O (measured 3.9e-3 vs f64 on
the real data; fp8 fails at 3.5e-2). x and out cross HBM as bf16 -> traffic
halves vs f32 (64.5 MiB/core, ~180 us at 358 GB/s/core).

PE shape: TRN2 PE runs f32r and bf16 matmuls at the same 1 col/cycle, so the
win over the f32 baseline comes from (a) removing all PE transposes and (b)
minimizing stationary reloads. Both sides of the matmul live in transposed
layout: host pre-transposes x per core to xT [512, 32768] bf16, and the
device computes outT [512, 32768] = wmodT.T-blocks @ xT, which the host
transposes back. The weight block [128i, 128o] is the stationary operand:
only 256 ldweights per core (vs 2048 transposes+loads in the f32 baseline);
matmul floor 1024 x 512 cols = 524288 cycles = 218 us @ 2.4 GHz.

Measured: 247.9 us on hw (f32 baseline: 378.8 us; PE floor for the 1024
512-col matmuls is ~230 us at the measured 225 ns/matmul including
ldweights+issue overhead). Rel err vs f64 reference: 3.87e-3.

Per core (rows = 32768), supers of R=2048 rows:
  for s in 16 supers:
    4x DMA xT[ic][:, s] -> SBUF [128, 4x2048] bf16 (4 KiB descriptors)
    for oc in 4:
      po [128 o, 2048 r] f32 PSUM (4 banks, double-buffered across oc)
      for ic in 4: ldweights wmodT[ic,oc]; 4x matmul 512-col rhs=xT slices
      evac ACT/DVE alternating: po + beta[oc] (per-partition bias) -> bf16
    1x DMA outT tile [128, 4x2048] -> DRAM (second HWDGE ring)
Sharding: x flat [B*N, IN] in 8 contiguous row blocks; batch boundary falls
between cores 3 and 4, so each core uses one (wmodT, beta) pair. No
cross-core communication.
"""

import numpy as np
import ml_dtypes

B, N = 2, 131072
IN_F, OUT_F, STYLE_F = 512, 512, 256
NCORES = 8
ROWS = B * N
ROWS_PER_CORE = ROWS // NCORES  # 32768
P = 128
R = 2048  # rows per superblock
BF16 = ml_dtypes.bfloat16


def _build_body(tc, out_ap, x_ap, wt_ap, betac_ap, rows_per_core):
    from concourse import mybir

    nc = tc.nc
    f32 = mybir.dt.float32
    bf16 = mybir.dt.bfloat16
    nsuper = rows_per_core // R  # 16
    NB = R // 512  # row-blocks (moving cols per matmul) per super

    with (
        tc.tile_pool(name="const", bufs=1) as cpool,
        tc.tile_pool(name="xin", bufs=2) as xpool,
        tc.tile_pool(name="oout", bufs=2) as opool,
        tc.tile_pool(name="pmm", bufs=2, space="PSUM") as pmpool,
    ):
        # Constants: wmodT blocks [128, (ic o)] and per-oc beta column [128, 4]
        wt_sb = cpool.tile([P, 4 * OUT_F], bf16)
        for ic in range(4):
            nc.sync.dma_start(
                out=wt_sb[:, ic * OUT_F : (ic + 1) * OUT_F],
                in_=wt_ap[ic * P : (ic + 1) * P, :],
            )
        betac_sb = cpool.tile([P, 4], f32)
        nc.sync.dma_start(out=betac_sb[:], in_=betac_ap[:, :])

        for s in range(nsuper):
            xt = xpool.tile([P, 4 * R], bf16)
            if s == 0:
                # Pipeline head: land the first halves of all 4 ic slabs
                # first, so the first oc's leading row-blocks can complete
                # their accumulation groups before the tail halves arrive.
                for hh in range(2):
                    for ic in range(4):
                        nc.sync.dma_start(
                            out=xt[:, ic * R + hh * (R // 2) : ic * R + (hh + 1) * (R // 2)],
                            in_=x_ap[
                                ic * P : (ic + 1) * P,
                                s * R + hh * (R // 2) : s * R + (hh + 1) * (R // 2),
                            ],
                        )
            else:
                for ic in range(4):
                    nc.sync.dma_start(
                        out=xt[:, ic * R : (ic + 1) * R],
                        in_=x_ap[ic * P : (ic + 1) * P, s * R : (s + 1) * R],
                    )

            ot = opool.tile([P, 4 * R], bf16)
            for oc in range(4):
                po = pmpool.tile([P, R], f32)
                for ic in range(4):
                    lhs = wt_sb[:, ic * OUT_F + oc * P : ic * OUT_F + (oc + 1) * P]
                    for rb in range(NB):
                        nc.tensor.matmul(
                            po[:, rb * 512 : (rb + 1) * 512],
                            lhs,
                            xt[:, ic * R + rb * 512 : ic * R + (rb + 1) * 512],
                            start=(ic == 0),
                            stop=(ic == 3),
                        )
                # Evacuate + beta (per-partition bias) + bf16 cast.
                # Alternate engines so two evacs run concurrently.
                if oc % 2 == 0:
                    nc.scalar.add(
                        out=ot[:, oc * R : (oc + 1) * R],
                        in_=po[:],
                        add=betac_sb[:, oc : oc + 1],
                    )
                else:
                    nc.vector.tensor_scalar_add(
                        out=ot[:, oc * R : (oc + 1) * R],
                        in0=po[:],
                        scalar1=betac_sb[:, oc : oc + 1],
                    )
                nc.scalar.dma_start(
                    out=out_ap[oc * P : (oc + 1) * P, s * R : (s + 1) * R],
                    in_=ot[:, oc * R : (oc + 1) * R],
                )


def build_nc(rows_per_core=ROWS_PER_CORE):
    import concourse.tile as tile
    from concourse import bacc, mybir

    f32 = mybir.dt.float32
    bf16 = mybir.dt.bfloat16
    nc = bacc.Bacc(
        "TRN2", target_bir_lowering=False, debug=False, num_devices=NCORES
    )
    x_t = nc.dram_tensor("x", [IN_F, rows_per_core], bf16, kind="ExternalInput")
    wt_t = nc.dram_tensor("wt", [IN_F, OUT_F], bf16, kind="ExternalInput")
    betac_t = nc.dram_tensor("betac", [P, 4], f32, kind="ExternalInput")
    out_t = nc.dram_tensor(
        "out", [OUT_F, rows_per_core], bf16, kind="ExternalOutput"
    )

    with tile.TileContext(nc) as tc:
        _build_body(
            tc, out_t.ap(), x_t.ap(), wt_t.ap(), betac_t.ap(), rows_per_core
        )
    nc.compile()
    return nc


_NC_CACHE = {}


def _get_nc(rows_per_core=ROWS_PER_CORE):
    if rows_per_core not in _NC_CACHE:
        _NC_CACHE[rows_per_core] = build_nc(rows_per_core)
    return _NC_CACHE[rows_per_core]


def host_prep(x, z, weight, weight_alpha, bias_alpha, weight_beta, bias_beta):
    """Per-batch modulated weights/biases + per-core transposed bf16 x."""
    z64 = z.astype(np.float64)
    alpha = (z64 @ weight_alpha.astype(np.float64).T) + bias_alpha.astype(np.float64)
    beta = (z64 @ weight_beta.astype(np.float64).T) + bias_beta.astype(np.float64)

    wmodT = [
        (weight.T.astype(np.float64) * alpha[b][:, None]).astype(BF16)
        for b in range(B)
    ]
    betac = [
        np.ascontiguousarray(
            beta[b].astype(np.float32).reshape(4, P).T
        )
        for b in range(B)
    ]

    xf = x.reshape(ROWS, IN_F)
    in_maps = []
    for k in range(NCORES):
        b = (k * ROWS_PER_CORE) // N
        xk = xf[k * ROWS_PER_CORE : (k + 1) * ROWS_PER_CORE]
        xT = np.ascontiguousarray(xk.astype(BF16).T)  # [512, 32768]
        in_maps.append({"x": xT, "wt": wmodT[b], "betac": betac[b]})
    return in_maps


def kernel(x, z, weight, weight_alpha, bias_alpha, weight_beta, bias_beta,
           _trace=False):
    from concourse.bass_utils import run_bass_kernel_spmd

    x = np.asarray(x, dtype=np.float32)
    z = np.asarray(z, dtype=np.float32)
    weight = np.asarray(weight, dtype=np.float32)
    weight_alpha = np.asarray(weight_alpha, dtype=np.float32)
    bias_alpha = np.asarray(bias_alpha, dtype=np.float32)
    weight_beta = np.asarray(weight_beta, dtype=np.float32)
    bias_beta = np.asarray(bias_beta, dtype=np.float32)
    in_maps = host_prep(
        x, z, weight, weight_alpha, bias_alpha, weight_beta, bias_beta
    )
    nc = _get_nc()
    res = run_bass_kernel_spmd(
        nc, in_maps, core_ids=list(range(NCORES)), trace=_trace
    )
    out = np.concatenate(
        [res.results[k]["out"].T.astype(np.float32) for k in range(NCORES)],
        axis=0,
    )
    out = out.reshape(B, N, OUT_F)
    if _trace:
        kernel.last_results = res
    return out


# revision 4
# speedup vs baseline: 1.0292x; 1.0032x over previous
"""Trainium2 Bass kernel for ModLinear forward — bf16, weight-stationary.

    alpha = z @ weight_alpha.T + bias_alpha          # [B, IN]
    beta  = z @ weight_beta.T  + bias_beta           # [B, OUT]
    out   = (x * alpha[:, None, :]) @ weight.T + beta[:, None, :]

alpha folds into the weight per batch: out[b] = x[b] @ wmodT[b] + beta[b]
with wmodT[b] = weight.T * alpha[b][:, None]  (tiny, host-computed).

Numerics: the 2e-2 rel-err gate admits bf16 I/O (measured 3.9e-3 vs f64 on
the real data; fp8 fails at 3.5e-2). x and out cross HBM as bf16 -> traffic
halves vs f32 (64.5 MiB/core, ~180 us at 358 GB/s/core).

PE shape: TRN2 PE runs f32r and bf16 matmuls at the same 1 col/cycle, so the
win over the f32 baseline comes from (a) removing all PE transposes and (b)
minimizing stationary reloads. Both sides of the matmul live in transposed
layout: host pre-transposes x per core to xT [512, 32768] bf16, and the
device computes outT [512, 32768] = wmodT.T-blocks @ xT, which the host
transposes back. The weight block [128i, 128o] is the stationary operand:
only 256 ldweights per core (vs 2048 transposes+loads in the f32 baseline);
matmul floor 1024 x 512 cols = 524288 cycles = 218 us @ 2.4 GHz.

Measured: 246.3 us on hw (f32 baseline: 378.8 us). Rel err vs f64
reference: 3.87e-3. PE floor for the 1024 512-col matmuls is ~223 us at
the measured ~218 ns/matmul steady-state rate; head/tail overlap
tricks below close most of the remaining gap.

Per core (rows = 32768), supers of R=2048 rows:
  for s in 16 supers:
    4x DMA xT[ic][:, s] -> SBUF [128, 4x2048] bf16 (4 KiB descriptors)
    for oc in 4:
      po [128 o, 2048 r] f32 PSUM (4 banks, double-buffered across oc)
      for ic in 4: ldweights wmodT[ic,oc]; 4x matmul 512-col rhs=xT slices
      evac ACT/DVE alternating: po + beta[oc] (per-partition bias) -> bf16
    1x DMA outT tile [128, 4x2048] -> DRAM (second HWDGE ring)
Sharding: x flat [B*N, IN] in 8 contiguous row blocks; batch boundary falls
between cores 3 and 4, so each core uses one (wmodT, beta) pair. No
cross-core communication.
"""

import numpy as np
import ml_dtypes

B, N = 2, 131072
IN_F, OUT_F, STYLE_F = 512, 512, 256
NCORES = 8
ROWS = B * N
ROWS_PER_CORE = ROWS // NCORES  # 32768
P = 128
R = 2048  # rows per superblock
BF16 = ml_dtypes.bfloat16


def _build_body(tc, out_ap, x_ap, wt_ap, betac_ap, rows_per_core):
    from concourse import mybir

    nc = tc.nc
    f32 = mybir.dt.float32
    bf16 = mybir.dt.bfloat16
    nsuper = rows_per_core // R  # 16
    NB = R // 512  # row-blocks (moving cols per matmul) per super

    with (
        tc.tile_pool(name="const", bufs=1) as cpool,
        tc.tile_pool(name="xin", bufs=2) as xpool,
        tc.tile_pool(name="oout", bufs=2) as opool,
        tc.tile_pool(name="pmm", bufs=2, space="PSUM") as pmpool,
    ):
        # Constants: wmodT blocks [128, (ic o)] and per-oc beta column [128, 4]
        wt_sb = cpool.tile([P, 4 * OUT_F], bf16)
        for ic in range(4):
            nc.sync.dma_start(
                out=wt_sb[:, ic * OUT_F : (ic + 1) * OUT_F],
                in_=wt_ap[ic * P : (ic + 1) * P, :],
            )
        betac_sb = cpool.tile([P, 4], f32)
        nc.sync.dma_start(out=betac_sb[:], in_=betac_ap[:, :])

        for s in range(nsuper):
            xt = xpool.tile([P, 4 * R], bf16)
            if s == 0:
                # Pipeline head: the store ring (scalar) is idle until the
                # first evac, so split the first super's four slabs across
                # both HWDGE rings -> the last slab lands in half the time.
                for ic in range(4):
                    eng = nc.sync if ic % 2 == 0 else nc.scalar
                    eng.dma_start(
                        out=xt[:, ic * R : (ic + 1) * R],
                        in_=x_ap[ic * P : (ic + 1) * P, s * R : (s + 1) * R],
                    )
            else:
                for ic in range(4):
                    nc.sync.dma_start(
                        out=xt[:, ic * R : (ic + 1) * R],
                        in_=x_ap[ic * P : (ic + 1) * P, s * R : (s + 1) * R],
                    )

            ot = opool.tile([P, 4 * R], bf16)
            for oc in range(4):
                po = pmpool.tile([P, R], f32)
                for ic in range(4):
                    lhs = wt_sb[:, ic * OUT_F + oc * P : ic * OUT_F + (oc + 1) * P]
                    for rb in range(NB):
                        nc.tensor.matmul(
                            po[:, rb * 512 : (rb + 1) * 512],
                            lhs,
                            xt[:, ic * R + rb * 512 : ic * R + (rb + 1) * 512],
                            start=(ic == 0),
                            stop=(ic == 3),
                        )
                # Evacuate + beta (per-partition bias) + bf16 cast.
                # Alternate engines so two evacs run concurrently. For the
                # very last (super, oc), chunk the drain in halves and use
                # the by-then-idle input ring for the final stores.
                last = s == nsuper - 1 and oc == 3
                nchunk = 2 if last else 1
                CW = R // nchunk
                for ch in range(nchunk):
                    osl = slice(oc * R + ch * CW, oc * R + (ch + 1) * CW)
                    if oc % 2 == 0:
                        nc.scalar.add(
                            out=ot[:, osl],
                            in_=po[:, ch * CW : (ch + 1) * CW],
                            add=betac_sb[:, oc : oc + 1],
                        )
                    else:
                        nc.vector.tensor_scalar_add(
                            out=ot[:, osl],
                            in0=po[:, ch * CW : (ch + 1) * CW],
                            scalar1=betac_sb[:, oc : oc + 1],
                        )
                    deng = nc.sync if last else nc.scalar
                    deng.dma_start(
                        out=out_ap[
                            oc * P : (oc + 1) * P,
                            s * R + ch * CW : s * R + (ch + 1) * CW,
                        ],
                        in_=ot[:, osl],
                    )


def build_nc(rows_per_core=ROWS_PER_CORE):
    import concourse.tile as tile
    from concourse import bacc, mybir

    f32 = mybir.dt.float32
    bf16 = mybir.dt.bfloat16
    nc = bacc.Bacc(
        "TRN2", target_bir_lowering=False, debug=False, num_devices=NCORES
    )
    x_t = nc.dram_tensor("x", [IN_F, rows_per_core], bf16, kind="ExternalInput")
    wt_t = nc.dram_tensor("wt", [IN_F, OUT_F], bf16, kind="ExternalInput")
    betac_t = nc.dram_tensor("betac", [P, 4], f32, kind="ExternalInput")
    out_t = nc.dram_tensor(
        "out", [OUT_F, rows_per_core], bf16, kind="ExternalOutput"
    )

    with tile.TileContext(nc) as tc:
        _build_body(
            tc, out_t.ap(), x_t.ap(), wt_t.ap(), betac_t.ap(), rows_per_core
        )
    nc.compile()
    return nc


_NC_CACHE = {}


def _get_nc(rows_per_core=ROWS_PER_CORE):
    if rows_per_core not in _NC_CACHE:
        _NC_CACHE[rows_per_core] = build_nc(rows_per_core)
    return _NC_CACHE[rows_per_core]


def host_prep(x, z, weight, weight_alpha, bias_alpha, weight_beta, bias_beta):
    """Per-batch modulated weights/biases + per-core transposed bf16 x."""
    z64 = z.astype(np.float64)
    alpha = (z64 @ weight_alpha.astype(np.float64).T) + bias_alpha.astype(np.float64)
    beta = (z64 @ weight_beta.astype(np.float64).T) + bias_beta.astype(np.float64)

    wmodT = [
        (weight.T.astype(np.float64) * alpha[b][:, None]).astype(BF16)
        for b in range(B)
    ]
    betac = [
        np.ascontiguousarray(
            beta[b].astype(np.float32).reshape(4, P).T
        )
        for b in range(B)
    ]

    xf = x.reshape(ROWS, IN_F)
    in_maps = []
    for k in range(NCORES):
        b = (k * ROWS_PER_CORE) // N
        xk = xf[k * ROWS_PER_CORE : (k + 1) * ROWS_PER_CORE]
        xT = np.ascontiguousarray(xk.astype(BF16).T)  # [512, 32768]
        in_maps.append({"x": xT, "wt": wmodT[b], "betac": betac[b]})
    return in_maps


def kernel(x, z, weight, weight_alpha, bias_alpha, weight_beta, bias_beta,
           _trace=False):
    from concourse.bass_utils import run_bass_kernel_spmd

    x = np.asarray(x, dtype=np.float32)
    z = np.asarray(z, dtype=np.float32)
    weight = np.asarray(weight, dtype=np.float32)
    weight_alpha = np.asarray(weight_alpha, dtype=np.float32)
    bias_alpha = np.asarray(bias_alpha, dtype=np.float32)
    weight_beta = np.asarray(weight_beta, dtype=np.float32)
    bias_beta = np.asarray(bias_beta, dtype=np.float32)
    in_maps = host_prep(
        x, z, weight, weight_alpha, bias_alpha, weight_beta, bias_beta
    )
    nc = _get_nc()
    res = run_bass_kernel_spmd(
        nc, in_maps, core_ids=list(range(NCORES)), trace=_trace
    )
    out = np.concatenate(
        [res.results[k]["out"].T.astype(np.float32) for k in range(NCORES)],
        axis=0,
    )
    out = out.reshape(B, N, OUT_F)
    if _trace:
        kernel.last_results = res
    return out
